# revision 8
# baseline (speedup 1.0000x reference)
"""MoE gate (DeepSeek-V2 style, group-limited greedy top-k) for Trainium2.

Full-input contract: kernel(hidden_states[4,8192,2048] f32, kernel[64,2048] f32)
-> topk_weight [32768, 6] f32.

Strategy: pure data-parallel over 8 NeuronCores (4096 tokens each).

Numerics: x is decomposed on-chip as x = a + r with a = f16(x) and
r = f16(x - a) (engine copy + subtract; the rounding mode of a is irrelevant
because r compensates exactly); w is decomposed once as w = u + s the same
way. Then three f16 matmul passes

    logitsT += uT.aT + uT.rT + sT.aT        (skipped term rT.sT ~ 2^-22)

reproduce fp32 logits to ~4e-6 absolute (measured on HW), far below the
~1e-5 group-selection margin of this input, while running at 1 cycle/row on
the PE -- and, unlike the f32r hi/lo scheme, the operands are 2-byte so the
transposition moves off the PE entirely:

Transposition: every [128t x 128h] chunk of a and r is transposed
SBUF->SBUF by the XBAR DMA-transpose engine on the SP/ACT HWDGE queues
(~56 ns of queue time per chunk in the cost model), eliminating both the PE
transpose passes and the PSUM->SBUF evictions of the f32r design.
(The gpsimd cast-DMA (f32->f16 straight from DRAM) would be cheaper still,
but on real HW an XBAR transpose reading an SBUF region written by a SWDGE
DMA races with it -- measured data corruption -- so a is engine-computed.)

Per core: tokens are remapped so partition p owns a contiguous 32-token DRAM
range (t = p*32 + m*4 + b), making every DMA descriptor large & contiguous.
Issue order is software-pipelined at megatile granularity:
fronts(m) [loads + a/r ops] -> matmuls(m-1) -> backs(m) [transposes] ->
post(m-1) [lts, logits back-transpose, top-k, store], so no engine FIFO
blocks on a dependency issued just before it.

Top-k per 128-token block on DVE/Pool/ACT using the hardware top-8 sort
(InstMax): softmax denominator cancels in the final normalization, so only
e = exp(logit - max) is needed; group-max -> sort -> 3rd value threshold ->
group mask -> masked e -> top-8 sort -> sum top-6 -> reciprocal -> scale.

Engine budget per core (cost model, 4096 tokens):
  PE   48 f16 mm/megatile + lg transposes + warmup ~87 us   <- critical
  SP   16 f32 loads + half the transposes + stores ~82 us
  ACT  24 a-ops + half transposes + exp/lts        ~84 us
  Pool 16 f32 loads + 4 a + 4 r + lsub/me          ~87 us
  DVE  28 r + 4 a + rest of top-k                  ~85 us
"""

import sys

if "/opt/trn_rl_repo" not in sys.path:
    sys.path.insert(0, "/opt/trn_rl_repo")

import numpy as np

# Problem constants (hardcoded per contract)
N_CORES = 8
H = 2048
E = 64  # n_routed_experts
G = 8  # n_group
PG = E // G  # experts per group
TG = 3  # topk_group
TK = 6  # top_k
P = 128  # partitions
MEGA = 512  # tokens per megatile
BB = MEGA // P  # 4 token blocks per megatile
KCH = H // P  # 16 contraction chunks


def build_nc(t_core, repeat=1):
    """Build the single-core Bass program for a t_core-token shard."""
    from concourse import bacc, mybir, masks
    from concourse.tile import TileContext

    f32 = mybir.dt.float32
    f16 = mybir.dt.float16
    X = mybir.AxisListType.X
    NM = t_core // MEGA
    assert t_core % MEGA == 0

    nc = bacc.Bacc()
    x = nc.declare_dram_parameter("x", [t_core, H], f32, isOutput=False)
    w = nc.declare_dram_parameter("w", [E, H], f32, isOutput=False)
    out = nc.declare_dram_parameter("out", [t_core, TK], f32, isOutput=True)

    with TileContext(nc) as tc:
        with (
            tc.tile_pool(name="const", bufs=1) as cpool,
            tc.tile_pool(name="xq", bufs=5) as xqpool,
            tc.tile_pool(name="aq", bufs=6) as aqpool,
            tc.tile_pool(name="at", bufs=2) as atpool,
            tc.tile_pool(name="rt", bufs=2) as rtpool,
            tc.tile_pool(name="lts", bufs=2) as ltspool,
            tc.tile_pool(name="small", bufs=2) as spool,
            tc.tile_pool(name="outp", bufs=2) as opool,
            tc.tile_pool(name="ps_mm", bufs=2, space="PSUM") as psmm,
            tc.tile_pool(name="ps_lg", bufs=2, space="PSUM") as pslg,
            tc.tile_pool(name="ps_wm", bufs=1, space="PSUM") as pswm,
        ):
            identf = cpool.tile([P, P], f32)
            masks.make_identity(nc, identf[:])
            idf = identf[:]

            w_sb = cpool.tile([E, H], f32)
            u_sb = cpool.tile([E, H], f16)
            s_sb = cpool.tile([E, H], f16)
            uT = cpool.tile([P, KCH, E], f16)
            sT = cpool.tile([P, KCH, E], f16)

            def warm_pe(n=26):
                # Dummy identity transposes burn through the PE p-state ramp
                # (~3us of continuous activity) during the otherwise PE-idle
                # DMA head, so real matmuls start at full clock.
                pwm = pswm.tile([P, P], f32, tag="wm")
                for _ in range(n):
                    nc.tensor.transpose(pwm[:], idf, idf)

            def setup_w():
                # w -> u = f16(w), s = f16(w - u); DMA-transpose both to
                # [128h, k, 64e]. Issued after megatile 0's fronts so it
                # doesn't gate the pipeline head.
                nc.sync.dma_start(out=w_sb[:], in_=w[:])
                nc.scalar.copy(u_sb[:], w_sb[:])
                nc.vector.tensor_tensor(
                    s_sb[:], w_sb[:], u_sb[:], mybir.AluOpType.subtract
                )
                for k in range(KCH):
                    nc.sync.dma_start(
                        out=uT[:, k, :],
                        in_=u_sb[:, k * P : (k + 1) * P],
                        transpose=True,
                    )
                    nc.scalar.dma_start(
                        out=sT[:, k, :],
                        in_=s_sb[:, k * P : (k + 1) * P],
                        transpose=True,
                    )

            xr = x[:].rearrange("(p m b) h -> p m b h", p=P, m=NM, b=BB)
            our = out[:].rearrange("(p m b) k -> p m b k", p=P, m=NM, b=BB)

            def front_megatile(m):
                # Per quarter b: f32 load (SP / ACT / Pool), a = f16(x)
                # (ACT / DVE / Pool), r = f16(x - a) (DVE / Pool), both into
                # one combined arq tile so a single fence covers them.
                quarters = []
                for b in range(BB):
                    xt = xqpool.tile([P, H], f32, tag="xq")
                    if b < 2:
                        load_eng = nc.sync
                    elif b == 2 or m % 4 < 3:
                        load_eng = nc.gpsimd
                    else:
                        load_eng = nc.scalar
                    load_eng.dma_start(out=xt[:], in_=xr[:, m, b, :])
                    arq = aqpool.tile([P, 2, H], f16, tag="arq")
                    if b < 2 or (b == 2 and m % 2 == 0):
                        nc.scalar.copy(arq[:, 0, :], xt[:])
                    elif b == 2 or m % 2 == 0:
                        eng = nc.vector if m % 2 == 0 else nc.gpsimd
                        eng.tensor_copy(arq[:, 0, :], xt[:])
                    else:
                        nc.gpsimd.tensor_copy(arq[:, 0, :], xt[:])
                    r_eng = nc.gpsimd if (b == 3 and m % 2 == 1) else nc.vector
                    r_eng.tensor_tensor(
                        arq[:, 1, :], xt[:], arq[:, 0, :],
                        mybir.AluOpType.subtract,
                    )
                    quarters.append(arq)
                return quarters

            def back_megatile(m, quarters):
                # 32 XBAR DMA transposes per quarter into aT/rT
                # [128h, k, 512t]; each quarter's transposes live on ONE
                # queue (alternating SP/ACT) behind a tiny front-fence DMA
                # that reads the arq tile. The fence waits on the same a/r
                # producer semaphores, and its own DMA-pipeline latency
                # guarantees the engine writes have settled in SBUF before
                # the XBAR reads start (engine-sem -> DMA-read settling race
                # corrupted data on real HW without it).
                at = atpool.tile([P, KCH, MEGA], f16, tag="at")
                rt = rtpool.tile([P, KCH, MEGA], f16, tag="rt")
                for b, arq in enumerate(quarters):
                    q_eng = nc.sync if (m * BB + b) % 2 == 0 else nc.scalar
                    fd = spool.tile([P, 2, 2], f16, tag=f"fd{(m * BB + b) % 2}")
                    q_eng.dma_start(out=fd[:], in_=arq[:, :, 0:2])
                    for k in range(KCH):
                        q_eng.dma_start(
                            out=at[:, k, b * P : (b + 1) * P],
                            in_=arq[:, 0, k * P : (k + 1) * P],
                            transpose=True,
                        )
                        q_eng.dma_start(
                            out=rt[:, k, b * P : (b + 1) * P],
                            in_=arq[:, 1, k * P : (k + 1) * P],
                            transpose=True,
                        )
                return at, rt

            def compute_mm(at, rt, t0=0, width=MEGA):
                # logitsT[64, width] += uT.aT + uT.rT + sT.aT (f16, 1 cyc/row)
                lt = psmm.tile([E, width], f32, tag="lt")
                n_acc = 3 * KCH
                i_acc = 0
                for k in range(KCH):
                    for wt_k, xt_k in ((uT, at), (uT, rt), (sT, at)):
                        nc.tensor.matmul(
                            lt[:],
                            wt_k[:, k, :],
                            xt_k[:, k, t0 : t0 + width],
                            start=(i_acc == 0),
                            stop=(i_acc == n_acc - 1),
                        )
                        i_acc += 1
                return lt

            def compute_post(m, lt, t0=0, width=MEGA):
                nb = width // P  # token blocks in this slice
                b0 = t0 // P
                lts = ltspool.tile([E, width], f32, tag="lts")
                nc.scalar.copy(lts[:], lt[:])

                # transpose logits back -> [128t, 64e] blocks in PSUM (fp32)
                lg = pslg.tile([P, nb * E], f32, tag="lg")
                for b in range(nb):
                    nc.tensor.transpose(
                        lg[:, b * E : (b + 1) * E],
                        lts[:, b * P : (b + 1) * P],
                        idf[0:E, 0:E],
                    )

                # --- top-k pipeline, all nb token-blocks fused per op ---
                lg3 = lg[:].rearrange("p (b e) -> p b e", b=nb)
                # e = exp(logit - max): keeps ACT exp args in [-24, 0] where
                # the table is ~4x more accurate (fewer selection-flip risks
                # near group-boundary ties). Per-block bias via DVE subtract.
                nmax = spool.tile([P, nb], f32, tag="nmax")
                nc.vector.tensor_reduce(
                    nmax[:], lg3, axis=X, op=mybir.AluOpType.max, negate=True
                )
                lsub = spool.tile([P, nb, E], f32, tag="lsub")
                nc.vector.tensor_tensor(
                    lsub[:],
                    lg3,
                    nmax[:].unsqueeze(2).broadcast_to([P, nb, E]),
                    mybir.AluOpType.add,
                )
                e_sb = spool.tile([P, nb, E], f32, tag="esb")
                nc.scalar.activation(
                    e_sb[:], lsub[:], mybir.ActivationFunctionType.Exp
                )
                e4 = e_sb[:].rearrange("p b (g j) -> p b g j", g=G)
                gmax = spool.tile([P, nb, G], f32, tag="gmax")
                nc.vector.tensor_reduce(
                    gmax[:], e4, axis=X, op=mybir.AluOpType.max
                )
                gsort = spool.tile([P, nb, 8], f32, tag="gsort")
                for b in range(nb):
                    nc.vector.max(gsort[:, b, :], gmax[:, b, :])
                gmask = spool.tile([P, nb, G], f32, tag="gmask")
                nc.vector.tensor_tensor(
                    gmask[:],
                    gmax[:],
                    gsort[:, :, TG - 1 : TG].broadcast_to([P, nb, G]),
                    mybir.AluOpType.is_ge,
                )
                me = spool.tile([P, nb, E], f32, tag="me")
                nc.gpsimd.tensor_tensor(
                    me[:].rearrange("p b (g j) -> p b g j", g=G),
                    e4,
                    gmask[:].unsqueeze(3).broadcast_to([P, nb, G, PG]),
                    mybir.AluOpType.mult,
                )
                t8 = spool.tile([P, nb, 8], f32, tag="t8")
                for b in range(nb):
                    nc.vector.max(t8[:, b, :], me[:, b, :])
                ssum = spool.tile([P, nb], f32, tag="ssum")
                nc.vector.tensor_reduce(
                    ssum[:], t8[:, :, 0:TK], axis=X, op=mybir.AluOpType.add
                )
                rec = spool.tile([P, nb], f32, tag="rec")
                nc.vector.reciprocal(rec[:], ssum[:])
                ow = opool.tile([P, nb, TK], f32, tag="ow")
                nc.vector.tensor_tensor(
                    ow[:],
                    t8[:, :, 0:TK],
                    rec[:].unsqueeze(2).broadcast_to([P, nb, TK]),
                    mybir.AluOpType.mult,
                )
                nc.sync.dma_start(out=our[:, m, b0 : b0 + nb], in_=ow[:])

            # software pipeline:
            #   fronts(m) -> matmuls(m-1) -> backs(m) -> post(m-1)
            prev = None  # (m, lt PSUM tile)
            prev_t = None  # (at, rt) awaiting matmul
            w_done = False
            warm_pe()
            for _r in range(repeat):
                for m in range(NM):
                    quarters = front_megatile(m)
                    if not w_done:
                        setup_w()
                        w_done = True
                    if prev_t is not None:
                        lt = compute_mm(*prev_t[1:])
                        prev = (prev_t[0], lt)
                    cur_t = (m, *back_megatile(m, quarters))
                    if prev is not None:
                        compute_post(*prev)
                        prev = None
                    prev_t = cur_t
            if prev_t is not None:
                # split the final megatile so its top-k overlaps the second
                # half-chain instead of serializing after the last matmul
                m_l, at_l, rt_l = prev_t
                lt1 = compute_mm(at_l, rt_l, 0, MEGA // 2)
                compute_post(m_l, lt1, 0, MEGA // 2)
                lt2 = compute_mm(at_l, rt_l, MEGA // 2, MEGA // 2)
                compute_post(m_l, lt2, MEGA // 2, MEGA // 2)

    nc.compile()
    return nc


_NC_CACHE = {}


def _get_nc(t_core):
    if t_core not in _NC_CACHE:
        _NC_CACHE[t_core] = build_nc(t_core)
    return _NC_CACHE[t_core]


def run_sharded(flat_x, w, trace=False, **kw):
    """flat_x: [T, H] f32. Returns ([T, 6] f32, BassKernelResults)."""
    from concourse.bass_utils import run_bass_kernel_spmd

    T = flat_x.shape[0]
    tc = T // N_CORES
    nc = _get_nc(tc)
    in_maps = [
        {"x": np.ascontiguousarray(flat_x[i * tc : (i + 1) * tc]), "w": w}
        for i in range(N_CORES)
    ]
    res = run_bass_kernel_spmd(nc, in_maps, list(range(N_CORES)), trace=trace, **kw)
    outs = [np.asarray(res.results[i]["out"]) for i in range(N_CORES)]
    return np.concatenate(outs, axis=0), res


def kernel(hidden_states, kernel):
    hs = np.asarray(hidden_states, dtype=np.float32)
    w = np.ascontiguousarray(np.asarray(kernel, dtype=np.float32))
    B, S, Hh = hs.shape
    flat = np.ascontiguousarray(hs.reshape(B * S, Hh))
    out, _ = run_sharded(flat, w)
    return out
